# revision 2
# baseline (speedup 1.0000x reference)
"""ACmix forward (nn_ACmix_58798102282697) on 8 Trainium2 NeuronCores.

Data-parallel over batch b=16 -> 2 samples per core; parameters replicated.
The whole forward is batch-local (the long/short-range attention batches are
within-sample and the grouped depthwise conv is per-sample), so batch
sharding needs no collectives. Compiled as two SPMD jit stages via the axon
PJRT plugin. The grouped depthwise 3x3 conv is expressed as 9 shift-and-add
einsums: lax.conv_general_dilated's lowering blows the NEFF instruction
limit and defeats the batch partitioner on this backend.
"""

import numpy as np
import jax
import jax.numpy as jnp
from jax.sharding import Mesh, NamedSharding, PartitionSpec as P

HEAD = 4   # n heads
KC = 3     # kernel_conv
DH = 8     # down_factor h
DW = 8     # down_factor w

N_CORES = 8


def _c1x1(x, w, b=None):
    y = jnp.einsum('bchw,oc->bohw', x, w)
    return y if b is None else y + b[None, :, None, None]


def _bn_relu(x, scale, shift):
    return jax.nn.relu(x * scale[None, :, None, None] + shift[None, :, None, None])


def _sa(fq, fk, fv, H, W):
    B, C = fq.shape[0], fq.shape[1]
    qf = fq.reshape(B, C, H * W)
    kf = fk.reshape(B, C, H * W)
    vf = fv.reshape(B, C, H * W)
    att = jax.nn.softmax(jnp.einsum('bcn,bcm->bnm', qf, kf) * (C ** -0.5), axis=-1)
    return jnp.einsum('bnm,bcm->bcn', att, vf).reshape(B, C, H, W)


def _stage_att(x, p):
    """qkv convs + long-range + short-range attention -> (out_att, q, k, v)."""
    b, _, h, w = x.shape
    C = p['conv1_w'].shape[0]
    hd = C // HEAD
    q = _c1x1(x, p['conv1_w'], p['conv1_b'])
    k = _c1x1(x, p['conv2_w'], p['conv2_b'])
    v = _c1x1(x, p['conv3_w'], p['conv3_b'])
    loc = jnp.stack([
        jnp.broadcast_to(jnp.linspace(-1.0, 1.0, w)[None, :], (h, w)),
        jnp.broadcast_to(jnp.linspace(-1.0, 1.0, h)[:, None], (h, w)),
    ], 0)[None]
    pos = jnp.tile(_c1x1(loc, p['convp_w'], p['convp_b']), (1, HEAD, 1, 1))
    fq = q * (float(hd) ** -0.5) + pos
    fk = k + pos
    oh, ow = h // DH, w // DW

    def blockify(t):
        return t.reshape(b, C, oh, DH, ow, DW).transpose(0, 3, 5, 1, 2, 4).reshape(
            b * DH * DW, C, oh, ow)

    ctx = _sa(blockify(fq), blockify(fk), blockify(v), oh, ow)
    feats = _bn_relu(_c1x1(ctx, p['lr_W_w']), p['lr_W_scale'], p['lr_W_shift'])
    feats = feats.reshape(b, DH, DW, C, oh, ow).transpose(0, 4, 5, 3, 1, 2).reshape(
        b * oh * ow, C, DH, DW)
    qx = _bn_relu(_c1x1(_bn_relu(_c1x1(feats, p['sr_fq1_w']), p['sr_fq1_scale'],
                                 p['sr_fq1_shift']), p['sr_fq2_w']),
                  p['sr_fq2_scale'], p['sr_fq2_shift'])
    kx = _bn_relu(_c1x1(_bn_relu(_c1x1(feats, p['sr_fk1_w']), p['sr_fk1_scale'],
                                 p['sr_fk1_shift']), p['sr_fk2_w']),
                  p['sr_fk2_scale'], p['sr_fk2_shift'])
    vx = _c1x1(feats, p['sr_fv_w'])
    ctx2 = _sa(qx, kx, vx, DH, DW)
    feats2 = _bn_relu(_c1x1(ctx2, p['sr_W_w']), p['sr_W_scale'], p['sr_W_shift'])
    out_att = feats2.reshape(b, oh, ow, C, DH, DW).transpose(0, 3, 1, 4, 2, 5).reshape(
        b, C, h, w)
    return out_att, q, k, v


def _stage_conv(out_att, q, k, v, p):
    """fc mixing + grouped depthwise 3x3 (as 9 shifted einsums) + combine."""
    b, C, h, w = q.shape
    hd = C // HEAD
    qh = q.reshape(b, HEAD, hd, h * w)
    kh = k.reshape(b, HEAD, hd, h * w)
    vh = v.reshape(b, HEAD, hd, h * w)
    # [b, 12, hd, n] x [9, 12] -> [b, 9, hd, n]
    f_all = jnp.einsum('bidn,oi->bodn', jnp.concatenate([qh, kh, vh], 1), p['fc_w'])
    f = f_all.reshape(b, KC * KC, hd, h, w)
    fp = jnp.pad(f, ((0, 0), (0, 0), (0, 0), (1, 1), (1, 1)))
    # dep_w [256, 9, 3, 3] -> [g=64, r=4, o=9, ky, kx]; group g == depth index d
    W = p['dep_w'].reshape(hd, C // hd, KC * KC, KC, KC)
    out_conv = jnp.zeros((b, hd, C // hd, h, w), jnp.float32)
    for ky in range(KC):
        for kx in range(KC):
            sl = fp[:, :, :, ky:ky + h, kx:kx + w]      # [b, 9, g, h, w]
            out_conv = out_conv + jnp.einsum(
                'bogyx,gro->bgryx', sl, W[:, :, :, ky, kx])
    out_conv = out_conv.reshape(b, C, h, w)
    return p['rate1'] * out_att + p['rate2'] * out_conv


_cache = {}


def _get_jitted():
    if 'f' not in _cache:
        devs = jax.devices()[:N_CORES]
        mesh = Mesh(np.array(devs), ('b',))
        xsh = NamedSharding(mesh, P('b'))
        rep = NamedSharding(mesh, P())
        f1 = jax.jit(_stage_att, in_shardings=(xsh, rep),
                     out_shardings=(xsh, xsh, xsh, xsh))
        f2 = jax.jit(_stage_conv, in_shardings=(xsh, xsh, xsh, xsh, rep),
                     out_shardings=xsh)
        _cache['f'] = (f1, f2, xsh, rep)
    return _cache['f']


def kernel(**inputs):
    x = np.ascontiguousarray(inputs['x'], dtype=np.float32)
    params = {k: np.asarray(v) for k, v in inputs.items() if k != 'x'}
    f1, f2, xsh, rep = _get_jitted()
    xd = jax.device_put(x, xsh)
    pd = jax.device_put(params, rep)
    out_att, q, k, v = f1(xd, pd)
    out = f2(out_att, q, k, v, pd)
    out.block_until_ready()
    return np.asarray(out).astype(np.float32)
